# revision 7
# baseline (speedup 1.0000x reference)
"""Trainium2 Bass kernel: low-rank (LoRA-style) linear with 2:4 soft-threshold
pruned weights, fp16 matmul / fp32 accumulate.

  wA = soft_threshold24(weight_A) * scale_A          # [IN, R]
  wB = soft_threshold24(weight_B) * scale_B          # [OUT, R]
  x_proj = f16(x) @ f16(wA)            (f32 accum)   # [N, R]
  out    = f16(x_proj) @ f16(wB).T + bias            # [N, OUT]

Sharding: data-parallel over the token dim across 8 cores (2048 tokens/core),
small weights replicated. No collectives.

The tiny weight tensors (wA/wB: [4096,64] each) are soft-thresholded, scaled,
cast and laid out on the host; the device kernel is a pure streaming pipeline:

  sync-DMA x f32 -> PE transpose (f32r) -> DVE/ACT copy PSUM->SBUF f16 ->
  32 accumulating f16 matmuls vs wA -> cast to f16 + ones row -> f16 matmuls
  vs wB.T (bias row folded in) -> DVE/ACT copy PSUM->SBUF -> DMA store.

The group pipeline is software-pipelined so the PE never waits on the small
x_proj copy: mm2 of group g is emitted after mm1 of group g+1.  PSUM->SBUF
copies are split across DVE and ACT; stores are chunked to avoid bursts.
"""

import sys

import numpy as np

if "/opt/trn_rl_repo" not in sys.path:
    sys.path.insert(0, "/opt/trn_rl_repo")

B, S, IN_F, OUT_F, RANK = 4, 4096, 4096, 4096, 64
N_CORES = 8
N_TOK = B * S                   # 16384
T_CORE = N_TOK // N_CORES       # 2048 tokens per core
P = 128
TT = 2                          # token tiles per group
GTOK = TT * P                   # 256 tokens per group
N_GRP = T_CORE // GTOK          # 8 groups per core
N_IB = IN_F // P                # 32 input-feature blocks
MM2_N = 512
N_OB = OUT_F // MM2_N           # 8 output column groups
ST_CH = 2                       # store chunks per token tile

_CACHE = {}


def _soft_threshold24_np(w):
    """Host-side 2:4 soft-threshold: keep top-2 |w| per group of 4 along the
    last dim, soft-threshold by the 3rd-largest |w|."""
    g = w.reshape(w.shape[0], -1, 4)
    a = np.abs(g)
    t = np.sort(a, axis=-1)[..., 1:2]
    s = np.sign(g) * np.maximum(a - t, 0.0)
    return s.reshape(w.shape).astype(np.float32)


def _build():
    import concourse.mybir as mybir
    import concourse.tile as tile
    from concourse import bacc
    from concourse.bass import ts

    f32, f16 = mybir.dt.float32, mybir.dt.float16
    f32r = mybir.dt.float32r

    nc = bacc.Bacc("TRN2", target_bir_lowering=False, debug=False,
                   enable_asserts=False)
    x_d = nc.dram_tensor("x", [T_CORE, IN_F], f32r, kind="ExternalInput")
    wa_d = nc.dram_tensor("wa", [P, N_IB * RANK], f16, kind="ExternalInput")
    wb_d = nc.dram_tensor("wbt", [RANK + 1, OUT_F], f16, kind="ExternalInput")
    id_d = nc.dram_tensor("ident", [P, P], f32r, kind="ExternalInput")
    o_d = nc.dram_tensor("out", [T_CORE, OUT_F], f32, kind="ExternalOutput")

    with tile.TileContext(nc) as tc:
        with (
            tc.tile_pool(name="const", bufs=1) as constp,
            tc.tile_pool(name="xin", bufs=7) as xin,
            tc.tile_pool(name="xtp", bufs=2) as xtp,
            tc.tile_pool(name="outp", bufs=2) as outp,
            tc.tile_pool(name="proj", bufs=2) as projp,
            tc.tile_pool(name="pst", bufs=3, space="PSUM") as pst,
            tc.tile_pool(name="ps1", bufs=2, space="PSUM") as ps1p,
            tc.tile_pool(name="ps2", bufs=3, space="PSUM") as ps2p,
        ):
            def cp_v(out, in_):
                nc.vector.tensor_copy(out=out, in_=in_)

            def cp_a(out, in_):
                nc.scalar.copy(out, in_)

            # weight/identity loads on the ACT DGE ring so the x loads on the
            # SP ring are not queued behind them
            ident = constp.tile([P, P], f32r)
            nc.scalar.dma_start(ident[:], id_d[:])
            wa_t = constp.tile([P, N_IB, RANK], f16)
            nc.scalar.dma_start(wa_t[:], wa_d[:].rearrange("p (b r) -> p b r",
                                                           b=N_IB))
            wbt = constp.tile([RANK + 1, OUT_F], f16)
            nc.scalar.dma_start(wbt[:], wb_d[:])

            def emit_front(g):
                """Loads, transposes, mm1 and x_proj cast for group g."""
                xts = []
                for tt in range(TT):
                    i = g * TT + tt
                    xt32 = xin.tile([P, IN_F], f32r, name="xt32", tag="xt32")
                    nc.sync.dma_start(xt32[:], x_d[ts(i, P), :])
                    xts.append(xt32)

                # transpose [t, in] -> [in, t] on PE; cast to f16 on the copy
                xT = xtp.tile([P, N_IB, GTOK], f16)
                for b in range(N_IB):
                    pt = pst.tile([P, GTOK], f32r, tag="pt", name="pt")
                    for tt in range(TT):
                        nc.tensor.transpose(pt[:, ts(tt, P)],
                                            xts[tt][:, ts(b, P)], ident[:])
                    cp = (cp_v, cp_a)[b % 2]
                    cp(xT[:, b, :], pt[:])

                # mm1: x_projT[r, t] = sum_i wa[i, r] * xT[i, t]  (f16)
                ps1 = ps1p.tile([RANK, GTOK], f32)
                for b in range(N_IB):
                    nc.tensor.matmul(ps1[:], wa_t[:, b, :], xT[:, b, :],
                                     start=(b == 0), stop=(b == N_IB - 1))

                xpa = projp.tile([RANK + 1, GTOK], f16)
                nc.vector.tensor_copy(out=xpa[0:RANK, :], in_=ps1[:])
                nc.vector.memset(xpa[RANK : RANK + 1, :], 1.0)
                return xpa

            def emit_back(g, xpa):
                """mm2, output copies and chunked stores for group g."""
                for tt in range(TT):
                    i = g * TT + tt
                    ob = outp.tile([P, OUT_F], f32, name="ob", tag="ob")
                    for j in range(N_OB):
                        ps2 = ps2p.tile([P, MM2_N], f32, tag="ps2", name="ps2")
                        nc.tensor.matmul(ps2[:], xpa[:, ts(tt, P)],
                                         wbt[:, ts(j, MM2_N)],
                                         start=True, stop=True)
                        cp = (cp_v, cp_a)[(j + tt) % 2]
                        cp(ob[:, ts(j, MM2_N)], ps2[:])
                        if (j + 1) % (N_OB // ST_CH) == 0:
                            c = j // (N_OB // ST_CH)
                            w = OUT_F // ST_CH
                            nc.sync.dma_start(o_d[ts(i, P), ts(c, w)],
                                              ob[:, ts(c, w)])

            xpa_prev = None
            for g in range(N_GRP):
                xpa = emit_front(g)
                if xpa_prev is not None:
                    emit_back(g - 1, xpa_prev)
                xpa_prev = xpa
            emit_back(N_GRP - 1, xpa_prev)

    nc.compile()
    return nc


def get_nc(*_args):
    if "nc" not in _CACHE:
        _CACHE["nc"] = _build()
    return _CACHE["nc"]


def prep_in_maps(x, weight_A, weight_B, bias, scale_A, scale_B):
    """Host-side prep: shard x, threshold/scale/cast/lay out the weights."""
    x = np.ascontiguousarray(np.asarray(x, dtype=np.float32))
    wa = np.asarray(weight_A, dtype=np.float32)
    wb = np.asarray(weight_B, dtype=np.float32)
    bi = np.asarray(bias, dtype=np.float32).reshape(OUT_F)
    sa = float(np.asarray(scale_A))
    sb = float(np.asarray(scale_B))

    # wa in PE-stationary layout: [P, N_IB * RANK], partition = row-in-block
    wa_p = (_soft_threshold24_np(wa) * sa).astype(np.float16)
    wa_p = np.ascontiguousarray(
        wa_p.reshape(N_IB, P, RANK).transpose(1, 0, 2).reshape(P, N_IB * RANK))
    wb_p = (_soft_threshold24_np(wb) * sb).astype(np.float16)   # [OUT, R]
    wbt = np.empty((RANK + 1, OUT_F), dtype=np.float16)
    wbt[0:RANK] = wb_p.T
    wbt[RANK] = bi.astype(np.float16)
    ident = np.eye(P, dtype=np.float32)

    xf = x.reshape(N_TOK, IN_F)
    return [
        {
            "x": xf[c * T_CORE : (c + 1) * T_CORE],
            "wa": wa_p,
            "wbt": wbt,
            "ident": ident,
        }
        for c in range(N_CORES)
    ]


def kernel(x, weight_A, weight_B, bias, scale_A, scale_B):
    from concourse.bass_utils import run_bass_kernel_spmd

    nc = get_nc()
    in_maps = prep_in_maps(x, weight_A, weight_B, bias, scale_A, scale_B)
    res = run_bass_kernel_spmd(nc, in_maps, core_ids=list(range(N_CORES)))
    out = np.concatenate([r["out"] for r in res.results], axis=0)
    return out.reshape(B, S, OUT_F)


# revision 9
# speedup vs baseline: 1.2037x; 1.2037x over previous
"""Trainium2 Bass kernel: low-rank (LoRA-style) linear with 2:4 soft-threshold
pruned weights, fp16 matmul / fp32 accumulate.

  wA = soft_threshold24(weight_A) * scale_A          # [IN, R]
  wB = soft_threshold24(weight_B) * scale_B          # [OUT, R]
  x_proj = f16(x) @ f16(wA)            (f32 accum)   # [N, R]
  out    = f16(x_proj) @ f16(wB).T + bias            # [N, OUT]

Sharding: data-parallel over the token dim across 8 cores (2048 tokens/core),
small weights replicated. No collectives.

The tiny weight tensors (wA/wB: [4096,64] each) are soft-thresholded, scaled,
cast and laid out on the host; the device kernel is a pure streaming pipeline:

  sync-DMA x f32 -> PE transpose (f32r) -> DVE/ACT copy PSUM->SBUF f16 ->
  32 accumulating f16 matmuls vs wA -> cast to f16 + ones row -> f16 matmuls
  vs wB.T (bias row folded in) -> DVE/ACT copy PSUM->SBUF -> DMA store.

The group pipeline is software-pipelined so the PE never waits on the small
x_proj copy: mm2 of group g is emitted after mm1 of group g+1.  PSUM->SBUF
copies are split across DVE and ACT; stores are chunked to avoid bursts.
"""

import sys

import numpy as np

if "/opt/trn_rl_repo" not in sys.path:
    sys.path.insert(0, "/opt/trn_rl_repo")

B, S, IN_F, OUT_F, RANK = 4, 4096, 4096, 4096, 64
N_CORES = 8
N_TOK = B * S                   # 16384
T_CORE = N_TOK // N_CORES       # 2048 tokens per core
P = 128
TT = 2                          # token tiles per group
GTOK = TT * P                   # 256 tokens per group
N_GRP = T_CORE // GTOK          # 8 groups per core
N_IB = IN_F // P                # 32 input-feature blocks
MM2_N = 512
N_OB = OUT_F // MM2_N           # 8 output column groups
ST_CH = 2                       # store chunks per token tile

_CACHE = {}


def _soft_threshold24_np(w):
    """Host-side 2:4 soft-threshold: keep top-2 |w| per group of 4 along the
    last dim, soft-threshold by the 3rd-largest |w|."""
    g = w.reshape(w.shape[0], -1, 4)
    a = np.abs(g)
    t = np.sort(a, axis=-1)[..., 1:2]
    s = np.sign(g) * np.maximum(a - t, 0.0)
    return s.reshape(w.shape).astype(np.float32)


def _build():
    import concourse.mybir as mybir
    import concourse.tile as tile
    from concourse import bacc
    from concourse.bass import ts

    f32, f16 = mybir.dt.float32, mybir.dt.float16
    f32r = mybir.dt.float32r

    nc = bacc.Bacc("TRN2", target_bir_lowering=False, debug=False,
                   enable_asserts=False)
    x_d = nc.dram_tensor("x", [T_CORE, IN_F], f32r, kind="ExternalInput")
    wa_d = nc.dram_tensor("wa", [P, N_IB * RANK], f16, kind="ExternalInput")
    wb_d = nc.dram_tensor("wbt", [RANK + 1, OUT_F], f16, kind="ExternalInput")
    id_d = nc.dram_tensor("ident", [P, P], f32r, kind="ExternalInput")
    o_d = nc.dram_tensor("out", [T_CORE, OUT_F], f32, kind="ExternalOutput")

    with tile.TileContext(nc) as tc:
        with (
            tc.tile_pool(name="const", bufs=1) as constp,
            tc.tile_pool(name="xin", bufs=7) as xin,
            tc.tile_pool(name="xtp", bufs=2) as xtp,
            tc.tile_pool(name="outp", bufs=2) as outp,
            tc.tile_pool(name="proj", bufs=2) as projp,
            tc.tile_pool(name="pst", bufs=3, space="PSUM") as pst,
            tc.tile_pool(name="ps1", bufs=1, space="PSUM") as ps1p,
            tc.tile_pool(name="ps2", bufs=4, space="PSUM") as ps2p,
        ):
            def cp_v(out, in_):
                nc.vector.tensor_copy(out=out, in_=in_)

            def cp_a(out, in_):
                nc.scalar.copy(out, in_)

            # weight/identity loads on the ACT DGE ring so the x loads on the
            # SP ring are not queued behind them
            ident = constp.tile([P, P], f32r)
            nc.scalar.dma_start(ident[:], id_d[:])
            wa_t = constp.tile([P, N_IB, RANK], f16)
            nc.scalar.dma_start(wa_t[:], wa_d[:].rearrange("p (b r) -> p b r",
                                                           b=N_IB))
            wbt = constp.tile([RANK + 1, OUT_F], f16)
            nc.scalar.dma_start(wbt[:], wb_d[:])

            def emit_front(g):
                """Loads, transposes, mm1 and x_proj cast for group g."""
                xts = []
                for tt in range(TT):
                    i = g * TT + tt
                    xt32 = xin.tile([P, IN_F], f32r, name="xt32", tag="xt32")
                    nc.sync.dma_start(xt32[:], x_d[ts(i, P), :])
                    xts.append(xt32)

                # transpose [t, in] -> [in, t] on PE; cast to f16 on the copy
                # (4 transposes = one full PSUM bank, drained by one copy)
                xT = xtp.tile([P, N_IB, GTOK], f16)
                for b2 in range(N_IB // 2):
                    pt = pst.tile([P, 2, GTOK], f32r, tag="pt", name="pt")
                    for q in range(2):
                        for tt in range(TT):
                            nc.tensor.transpose(pt[:, q, ts(tt, P)],
                                                xts[tt][:, ts(2 * b2 + q, P)],
                                                ident[:])
                    cp = (cp_v, cp_a)[b2 % 2]
                    cp(xT[:, 2 * b2 : 2 * b2 + 2, :], pt[:])

                # mm1: x_projT[r, t] = sum_i wa[i, r] * xT[i, t]  (f16)
                # reversed block order: the first mm1 needs the LAST xT copy,
                # so the scheduler cannot interleave mm1s into the transpose
                # stream (PE mode switches break the HAM warm-up)
                ps1 = ps1p.tile([RANK, GTOK], f32)
                for k, b in enumerate(reversed(range(N_IB))):
                    nc.tensor.matmul(ps1[:], wa_t[:, b, :], xT[:, b, :],
                                     start=(k == 0), stop=(k == N_IB - 1))

                xpa = projp.tile([RANK + 1, GTOK], f16)
                nc.vector.tensor_copy(out=xpa[0:RANK, :], in_=ps1[:])
                nc.vector.memset(xpa[RANK : RANK + 1, :], 1.0)
                return xpa

            def emit_back(g, xpa):
                """mm2, output copies and chunked stores for group g."""
                for tt in range(TT):
                    i = g * TT + tt
                    ob = outp.tile([P, OUT_F], f32, name="ob", tag="ob")
                    for j in range(N_OB):
                        ps2 = ps2p.tile([P, MM2_N], f32, tag="ps2", name="ps2")
                        nc.tensor.matmul(ps2[:], xpa[:, ts(tt, P)],
                                         wbt[:, ts(j, MM2_N)],
                                         start=True, stop=True)
                        cp = (cp_v, cp_a)[(j + tt) % 2]
                        cp(ob[:, ts(j, MM2_N)], ps2[:])
                        if (j + 1) % (N_OB // ST_CH) == 0:
                            c = j // (N_OB // ST_CH)
                            w = OUT_F // ST_CH
                            nc.sync.dma_start(o_d[ts(i, P), ts(c, w)],
                                              ob[:, ts(c, w)])

            xpa_prev = None
            for g in range(N_GRP):
                xpa = emit_front(g)
                if xpa_prev is not None:
                    emit_back(g - 1, xpa_prev)
                xpa_prev = xpa
            emit_back(N_GRP - 1, xpa_prev)

    nc.compile()
    return nc


def get_nc(*_args):
    if "nc" not in _CACHE:
        _CACHE["nc"] = _build()
    return _CACHE["nc"]


def prep_in_maps(x, weight_A, weight_B, bias, scale_A, scale_B):
    """Host-side prep: shard x, threshold/scale/cast/lay out the weights."""
    x = np.ascontiguousarray(np.asarray(x, dtype=np.float32))
    wa = np.asarray(weight_A, dtype=np.float32)
    wb = np.asarray(weight_B, dtype=np.float32)
    bi = np.asarray(bias, dtype=np.float32).reshape(OUT_F)
    sa = float(np.asarray(scale_A))
    sb = float(np.asarray(scale_B))

    # wa in PE-stationary layout: [P, N_IB * RANK], partition = row-in-block
    wa_p = (_soft_threshold24_np(wa) * sa).astype(np.float16)
    wa_p = np.ascontiguousarray(
        wa_p.reshape(N_IB, P, RANK).transpose(1, 0, 2).reshape(P, N_IB * RANK))
    wb_p = (_soft_threshold24_np(wb) * sb).astype(np.float16)   # [OUT, R]
    wbt = np.empty((RANK + 1, OUT_F), dtype=np.float16)
    wbt[0:RANK] = wb_p.T
    wbt[RANK] = bi.astype(np.float16)
    ident = np.eye(P, dtype=np.float32)

    xf = x.reshape(N_TOK, IN_F)
    return [
        {
            "x": xf[c * T_CORE : (c + 1) * T_CORE],
            "wa": wa_p,
            "wbt": wbt,
            "ident": ident,
        }
        for c in range(N_CORES)
    ]


def kernel(x, weight_A, weight_B, bias, scale_A, scale_B):
    from concourse.bass_utils import run_bass_kernel_spmd

    nc = get_nc()
    in_maps = prep_in_maps(x, weight_A, weight_B, bias, scale_A, scale_B)
    res = run_bass_kernel_spmd(nc, in_maps, core_ids=list(range(N_CORES)))
    out = np.concatenate([r["out"] for r in res.results], axis=0)
    return out.reshape(B, S, OUT_F)


# revision 14
# speedup vs baseline: 1.3542x; 1.1250x over previous
"""Trainium2 Bass kernel: low-rank (LoRA-style) linear with 2:4 soft-threshold
pruned weights, fp16 matmul / fp32 accumulate.

  wA = soft_threshold24(weight_A) * scale_A          # [IN, R]
  wB = soft_threshold24(weight_B) * scale_B          # [OUT, R]
  x_proj = f16(x) @ f16(wA)            (f32 accum)   # [N, R]
  out    = f16(x_proj) @ f16(wB).T + bias            # [N, OUT]

Sharding: data-parallel over the token dim across 8 cores (2048 tokens/core),
small weights replicated. No collectives.

The tiny weight tensors (wA/wB: [4096,64] each) are soft-thresholded, scaled,
cast and laid out on the host; the device kernel is a pure streaming pipeline:

  sync-DMA x f32 -> PE transpose (f32r) -> DVE/ACT copy PSUM->SBUF f16 ->
  32 accumulating f16 matmuls vs wA -> cast to f16 + ones row -> f16 matmuls
  vs wB.T (bias row folded in) -> DVE/ACT copy PSUM->SBUF -> DMA store.

The group pipeline is software-pipelined so the PE never waits on the small
x_proj copy: mm2 of group g is emitted after mm1 of group g+1.  PSUM->SBUF
copies are split across DVE and ACT; stores are chunked to avoid bursts.
"""

import sys

import numpy as np

if "/opt/trn_rl_repo" not in sys.path:
    sys.path.insert(0, "/opt/trn_rl_repo")

B, S, IN_F, OUT_F, RANK = 4, 4096, 4096, 4096, 64
N_CORES = 8
N_TOK = B * S                   # 16384
T_CORE = N_TOK // N_CORES       # 2048 tokens per core
P = 128
TT = 2                          # token tiles per group
GTOK = TT * P                   # 256 tokens per group
N_GRP = T_CORE // GTOK          # 8 groups per core
N_IB = IN_F // P                # 32 input-feature blocks
MM2_N = 512
N_OB = OUT_F // MM2_N           # 8 output column groups
ST_CH = 4                       # store chunks per token tile
XH = 2                          # x load chunks per token tile
IN_H = IN_F // XH               # 2048 features per load chunk

_CACHE = {}


def _soft_threshold24_np(w):
    """Host-side 2:4 soft-threshold: keep top-2 |w| per group of 4 along the
    last dim, soft-threshold by the 3rd-largest |w|."""
    g = w.reshape(w.shape[0], -1, 4)
    a = np.abs(g)
    t = np.sort(a, axis=-1)[..., 1:2]
    s = np.sign(g) * np.maximum(a - t, 0.0)
    return s.reshape(w.shape).astype(np.float32)


def _build():
    import concourse.mybir as mybir
    import concourse.tile as tile
    from concourse import bacc
    from concourse.bass import ts

    f32, f16 = mybir.dt.float32, mybir.dt.float16
    f32r = mybir.dt.float32r

    nc = bacc.Bacc("TRN2", target_bir_lowering=False, debug=False,
                   enable_asserts=False)
    x_d = nc.dram_tensor("x", [T_CORE, IN_F], f32r, kind="ExternalInput")
    wa_d = nc.dram_tensor("wa", [P, N_IB * RANK], f16, kind="ExternalInput")
    wb_d = nc.dram_tensor("wbt", [RANK + 1, OUT_F], f16, kind="ExternalInput")
    id_d = nc.dram_tensor("ident", [P, P], f32r, kind="ExternalInput")
    o_d = nc.dram_tensor("out", [T_CORE, OUT_F], f32, kind="ExternalOutput")

    with tile.TileContext(nc) as tc:
        with (
            tc.tile_pool(name="const", bufs=1) as constp,
            tc.tile_pool(name="xin", bufs=14) as xin,
            tc.tile_pool(name="xtp", bufs=2) as xtp,
            tc.tile_pool(name="outp", bufs=2) as outp,
            tc.tile_pool(name="proj", bufs=2) as projp,
            tc.tile_pool(name="pst", bufs=3, space="PSUM") as pst,
            tc.tile_pool(name="ps1", bufs=1, space="PSUM") as ps1p,
            tc.tile_pool(name="ps2", bufs=4, space="PSUM") as ps2p,
        ):
            def cp_v(out, in_):
                nc.vector.tensor_copy(out=out, in_=in_)

            def cp_a(out, in_):
                nc.scalar.copy(out, in_)

            # weight/identity loads on the ACT DGE ring so the x loads on the
            # SP ring are not queued behind them
            ident = constp.tile([P, P], f32r)
            nc.scalar.dma_start(ident[:], id_d[:])
            wa_t = constp.tile([P, N_IB, RANK], f16)
            nc.scalar.dma_start(wa_t[:], wa_d[:].rearrange("p (b r) -> p b r",
                                                           b=N_IB))
            wbt = constp.tile([RANK + 1, OUT_F], f16)
            nc.scalar.dma_start(wbt[:], wb_d[:])

            def emit_front(g):
                """Loads, transposes, mm1 and x_proj cast for group g."""
                xts = []
                for tt in range(TT):
                    i = g * TT + tt
                    halves = []
                    for h in range(XH):
                        xh = xin.tile([P, IN_H], f32r, name="xh", tag="xh")
                        nc.sync.dma_start(xh[:], x_d[ts(i, P), ts(h, IN_H)])
                        halves.append(xh)
                    xts.append(halves)

                # transpose [t, in] -> [in, t] on PE; cast to f16 on the copy
                # (4 transposes = one full PSUM bank, drained by one copy)
                xT = xtp.tile([P, N_IB, GTOK], f16)
                nbh = IN_H // P
                for b2 in range(N_IB // 2):
                    pt = pst.tile([P, 2, GTOK], f32r, tag="pt", name="pt")
                    for q in range(2):
                        b = 2 * b2 + q
                        for tt in range(TT):
                            nc.tensor.transpose(pt[:, q, ts(tt, P)],
                                                xts[tt][b // nbh][:, ts(b % nbh, P)],
                                                ident[:])
                    cp = (cp_v, cp_a)[b2 % 2]
                    cp(xT[:, 2 * b2 : 2 * b2 + 2, :], pt[:])

                # mm1: x_projT[r, t] = sum_i wa[i, r] * xT[i, t]  (f16)
                # reversed block order: the first mm1 needs the LAST xT copy,
                # so the scheduler cannot interleave mm1s into the transpose
                # stream (PE mode switches break the HAM warm-up)
                ps1 = ps1p.tile([RANK, GTOK], f32)
                for k, b in enumerate(reversed(range(N_IB))):
                    nc.tensor.matmul(ps1[:], wa_t[:, b, :], xT[:, b, :],
                                     start=(k == 0), stop=(k == N_IB - 1))

                xpa = projp.tile([RANK + 1, GTOK], f16)
                nc.vector.tensor_copy(out=xpa[0:RANK, :], in_=ps1[:])
                nc.vector.memset(xpa[RANK : RANK + 1, :], 1.0)
                return xpa

            def emit_back(g, xpa):
                """mm2, output copies and chunked stores for group g."""
                for tt in range(TT):
                    i = g * TT + tt
                    ob = outp.tile([P, OUT_F], f32, name="ob", tag="ob")
                    for j in range(N_OB):
                        ps2 = ps2p.tile([P, MM2_N], f32, tag="ps2", name="ps2")
                        nc.tensor.matmul(ps2[:], xpa[:, ts(tt, P)],
                                         wbt[:, ts(j, MM2_N)],
                                         start=True, stop=True)
                        cp = (cp_v, cp_a)[(j + tt) % 2]
                        cp(ob[:, ts(j, MM2_N)], ps2[:])
                        if (j + 1) % (N_OB // ST_CH) == 0:
                            c = j // (N_OB // ST_CH)
                            w = OUT_F // ST_CH
                            nc.sync.dma_start(o_d[ts(i, P), ts(c, w)],
                                              ob[:, ts(c, w)])

            xpa_prev = None
            for g in range(N_GRP):
                xpa = emit_front(g)
                if xpa_prev is not None:
                    emit_back(g - 1, xpa_prev)
                xpa_prev = xpa
            emit_back(N_GRP - 1, xpa_prev)

    nc.compile()
    return nc


def get_nc(*_args):
    if "nc" not in _CACHE:
        _CACHE["nc"] = _build()
    return _CACHE["nc"]


def prep_in_maps(x, weight_A, weight_B, bias, scale_A, scale_B):
    """Host-side prep: shard x, threshold/scale/cast/lay out the weights."""
    x = np.ascontiguousarray(np.asarray(x, dtype=np.float32))
    wa = np.asarray(weight_A, dtype=np.float32)
    wb = np.asarray(weight_B, dtype=np.float32)
    bi = np.asarray(bias, dtype=np.float32).reshape(OUT_F)
    sa = float(np.asarray(scale_A))
    sb = float(np.asarray(scale_B))

    # wa in PE-stationary layout: [P, N_IB * RANK], partition = row-in-block
    wa_p = (_soft_threshold24_np(wa) * sa).astype(np.float16)
    wa_p = np.ascontiguousarray(
        wa_p.reshape(N_IB, P, RANK).transpose(1, 0, 2).reshape(P, N_IB * RANK))
    wb_p = (_soft_threshold24_np(wb) * sb).astype(np.float16)   # [OUT, R]
    wbt = np.empty((RANK + 1, OUT_F), dtype=np.float16)
    wbt[0:RANK] = wb_p.T
    wbt[RANK] = bi.astype(np.float16)
    ident = np.eye(P, dtype=np.float32)

    xf = x.reshape(N_TOK, IN_F)
    return [
        {
            "x": xf[c * T_CORE : (c + 1) * T_CORE],
            "wa": wa_p,
            "wbt": wbt,
            "ident": ident,
        }
        for c in range(N_CORES)
    ]


def kernel(x, weight_A, weight_B, bias, scale_A, scale_B):
    from concourse.bass_utils import run_bass_kernel_spmd

    nc = get_nc()
    in_maps = prep_in_maps(x, weight_A, weight_B, bias, scale_A, scale_B)
    res = run_bass_kernel_spmd(nc, in_maps, core_ids=list(range(N_CORES)))
    out = np.concatenate([r["out"] for r in res.results], axis=0)
    return out.reshape(B, S, OUT_F)
